# revision 38
# baseline (speedup 1.0000x reference)
"""Multi-head causal attention (B=4,S=1024,D=768,H=12,Dh=64) on 8 trn2 cores.

Sharding: core c handles batch b=c//2 and the 6 heads hs=(c%2)*6 .. hs+6
(head-axis tensor parallel x batch parallel; 8 cores = 4 batches x 2 head-halves).

Per-core on-chip dataflow (bf16 matmul operands, fp32 PSUM accumulation):
  xT [768,1024] (host-pretransposed bf16), W{q,k,v} packed [6kc,128,3,384] bf16
  qT/kT = W-chunk.T(lhsT) @ xT    -> [64,1024] per head (transposed layout)
  v     = xT-chunk.T @ Wv          -> [1024, 6*65] per t-chunk (65th col = ones)
  scoresT[t,s] computed in 9 "bins" of 512 cols/head, each a [128,2,512] PSUM
  tile (head A bank 0, head B bank 1), double-buffered so the ScalarE Exp of
  bin k overlaps the PE scores of bin k+1.  Only causal-relevant pieces are
  computed; the two heads' K=64 score matmuls are emitted adjacently (PE row
  groups 0/64, different banks) so they execute concurrently on the array.
  Diagonal pieces land in bins 0-4 and are masked by [128,128] DVE multiplies
  with tri01.  PE idle gaps are filled with independent proj/v/ctx matmuls
  (engines run their streams in order, so the exp-paced scores chain must
  have filler work interleaved into the PE stream); sacrificial warm-up
  matmuls cover the HBM-bound input-DMA window to keep the PE clock ramped.
  ctxT_aug[65, s] = sum_j v_aug_j(lhsT) @ expT_j  (row 64 = softmax denom)
  y[65, h, s] staged bf16, DMA'd out; host divides by denominators+transposes.
"""

import threading
from contextlib import ExitStack

import ml_dtypes
import numpy as np

import concourse.bass as bass
import concourse.tile as tile
from concourse import bacc, mybir
from concourse.bass_utils import run_bass_kernel_spmd

B, S, D, H, DH = 4, 1024, 768, 12, 64
NCORES = 8
HL = H // 2          # 6 local heads per core
KC = D // 128        # 6 contraction chunks
NPAIR = HL // 2      # head pairs
F32 = mybir.dt.float32
BF16 = mybir.dt.bfloat16
N_WARM = 32

# ---- scores bin table -------------------------------------------------------
# A piece (j, s0, w, o) is the scoresT region for t-chunk j (psum partitions =
# t rel.), s in [s0, s0+w), placed at column o of its 512-col bin.  Bin b's
# flat exp-buffer base is 512*b.  Chunks are packed unsplit; all 8 diagonal
# chunks land in bins 0-4 so causal masking finishes early, and bin 8 holds a
# non-diag chunk so the final ctx piece follows bin 8's exp with no mask step.
BINS = [
    [(0, 0, 512, 0)],
    [(1, 128, 384, 0), (3, 384, 128, 384)],
    [(2, 256, 256, 0), (6, 768, 256, 256)],
    [(4, 512, 512, 0)],
    [(5, 640, 384, 0), (7, 896, 128, 384)],
    [(0, 512, 512, 0)],
    [(1, 512, 512, 0)],
    [(2, 512, 512, 0)],
    [(3, 512, 512, 0)],
]
EXP_COLS = 512 * len(BINS)  # 4608

# flat offset of each diagonal chunk j (mask = its first 128 cols), and the
# bin whose emission makes the mask legal to emit: group A after bin 2,
# group B after bin 4.
DIAG_OFF = {}
DIAG_BIN = {}
for _b, _pieces in enumerate(BINS):
    for (_j, _s0, _w, _o) in _pieces:
        if _s0 == 128 * _j:
            DIAG_OFF[_j] = 512 * _b + _o
            DIAG_BIN[_j] = _b
# group A (after bin 2): the c0 diagonals ctx-c0 waits on; group B (after
# bin 4): the rest.  All referenced exps are emitted by the group's bin.
MASK_GROUPS = {2: [0, 1, 2, 3], 4: [4, 5, 6, 7]}
for _g, _js in MASK_GROUPS.items():
    for _j in _js:
        assert DIAG_BIN[_j] <= _g, (_j, _g)


def _ctx_pieces():
    """Per output half c, pieces (j, s0, w, flat_off) to accumulate.
    c1 is ordered by flat offset = bin order, so when the unit is emitted
    (after bin 8) only its final piece still waits on an exp in flight."""
    halves = {0: [], 1: []}
    for b, pieces in enumerate(BINS):
        for (j, s0, w, o) in pieces:
            halves[s0 // 512].append((j, s0, w, 512 * b + o))
    c0 = sorted(halves[0], key=lambda p: p[0])
    c1 = sorted(halves[1], key=lambda p: p[3])
    return {0: c0, 1: c1}


CTX = _ctx_pieces()


def _emit_kernel(ctx: ExitStack, tc: tile.TileContext, xT, wqkv, im, y):
    nc = tc.nc

    const = ctx.enter_context(tc.tile_pool(name="const", bufs=1))
    im_sb = const.tile([128, 128], BF16)   # tri01: 1 where s_rel >= t_rel
    wsrc = const.tile([128, 128], BF16)
    nc.vector.memset(wsrc, 0.0)

    qk_pool = ctx.enter_context(tc.tile_pool(name="qk", bufs=1))
    qT = qk_pool.tile([128, NPAIR, S], BF16)  # partitions: (h%2)*64+e
    kT = qk_pool.tile([128, NPAIR, S], BF16)
    v_sb = qk_pool.tile([128, 8, HL * (DH + 1)], BF16)
    v4 = v_sb.rearrange("p j (h x) -> p j h x", h=HL)

    xtw = ctx.enter_context(tc.tile_pool(name="xtw", bufs=1))
    xt = xtw.tile([128, KC, S], BF16)
    w_all = xtw.tile([128, KC, 3, HL * DH], BF16)

    # PSUM budget (8 banks): pj0+pj1 (proj) 2, sg 2x2 (scores, double-buffered)
    # 4, cx0+cx1 (v proj + ctx) 2.
    pj = ctx.enter_context(tc.tile_pool(name="pj", bufs=1, space="PSUM"))
    sg = ctx.enter_context(tc.tile_pool(name="sg", bufs=2, space="PSUM"))
    cx = ctx.enter_context(tc.tile_pool(name="cx", bufs=1, space="PSUM"))
    ex = ctx.enter_context(tc.tile_pool(name="ex", bufs=3))
    yst = ctx.enter_context(tc.tile_pool(name="yst", bufs=3))

    # PE warm-up from a memset tile (no DMA dependency): opens the HAM clock
    # gate while input DMAs are still landing.
    warm = pj.tile([128, 128], F32, tag="pj0", name="warm")
    for i in range(N_WARM):
        nc.tensor.matmul(out=warm, lhsT=wsrc, rhs=wsrc,
                         start=(i == 0), stop=(i == N_WARM - 1))

    # Input DMAs: per contraction chunk, spread across the three DMA-capable
    # issue queues so chunk 0 lands fast and later chunks stagger in behind
    # the proj units (issue is ~0.8us of sequencer time per dma_start).
    def xt_dma(kc):
        return lambda eng: eng.dma_start(
            out=xt[:, kc, :], in_=xT[kc * 128:(kc + 1) * 128, :])

    def w_dma(kc):
        return lambda eng: eng.dma_start(out=w_all[:, kc, :, :], in_=wqkv[kc])

    # chunk-ascending rounds: proj unit [kc,kc+1] data lands just in time;
    # im (mask) is only needed ~15us later, so it issues after the inputs.
    for eng, issues in (
        (nc.sync, [xt_dma(0), xt_dma(1), w_dma(3), w_dma(4)]),
        (nc.scalar, [w_dma(0), xt_dma(2), xt_dma(3), w_dma(5)]),
        (nc.gpsimd, [w_dma(1), w_dma(2), xt_dma(4), xt_dma(5)]),
    ):
        for issue in issues:
            issue(eng)
    nc.sync.dma_start(out=im_sb, in_=im[:, :])
    nc.gpsimd.memset(v4[:, :, :, DH:DH + 1], 1.0)

    # ---- filler machinery: the PE stream is in-order, so the exp-paced
    # scores bins need independent matmul units interleaved between them.
    fillers = []  # list of (est_ns, kind, emit_fn)

    def emit_fillers(budget_ns):
        while fillers and budget_ns > 0:
            est, _, fn = fillers.pop(0)
            fn()
            budget_ns -= est

    def proj_qk_unit(pp, which, kcs, pool=None, tags=("pj0", "pj1")):
        """q/k projection for pair pp, contraction chunks kcs (accumulating)."""
        dst = (qT, kT)[which]
        pl = pool if pool is not None else pj

        def emit():
            pss = [pl.tile([128, 512], F32, tag=tags[i],
                           name=f"pp{pp}{which}{i}") for i in range(2)]
            for kc in kcs:
                for i, ps in enumerate(pss):
                    nc.tensor.matmul(
                        out=ps,
                        lhsT=w_all[:, kc, which, pp * 128:(pp + 1) * 128],
                        rhs=xt[:, kc, i * 512:(i + 1) * 512],
                        start=(kc == 0), stop=(kc == KC - 1))
            if kcs[-1] == KC - 1:
                for i, ps in enumerate(pss):
                    nc.vector.tensor_copy(
                        out=dst[:, pp, i * 512:(i + 1) * 512], in_=ps)
        def emit_final():
            # last-chunk unit, bank-major: each bank's cast is emitted right
            # after that bank's stop so the first cast starts half a unit early
            pss = [pl.tile([128, 512], F32, tag=tags[i],
                           name=f"pp{pp}{which}{i}") for i in range(2)]
            for i, ps in enumerate(pss):
                for kc in kcs:
                    nc.tensor.matmul(
                        out=ps,
                        lhsT=w_all[:, kc, which, pp * 128:(pp + 1) * 128],
                        rhs=xt[:, kc, i * 512:(i + 1) * 512],
                        start=False, stop=(kc == KC - 1))
                nc.vector.tensor_copy(
                    out=dst[:, pp, i * 512:(i + 1) * 512], in_=ps)
        return (900, "proj", emit_final if kcs[-1] == KC - 1 else emit)

    def proj_pair_units(pp):
        return [proj_qk_unit(pp, w, kcs)
                for w in (0, 1) for kcs in ([0, 1], [2, 3], [4, 5])]

    psv_tiles = {}

    def v_part(j, kcs, pool=None, tag=None):
        def emit():
            if j not in psv_tiles:
                pl = pool if pool is not None else cx
                psv_tiles[j] = pl.tile([128, HL * DH], F32,
                                       tag=tag or f"cx{j % 2}", name=f"psv{j}")
            psv = psv_tiles[j]
            for kc in kcs:
                nc.tensor.matmul(
                    out=psv,
                    lhsT=xt[:, kc, j * 128:(j + 1) * 128],
                    rhs=w_all[:, kc, 2, :],
                    start=(kc == 0), stop=(kc == KC - 1))
            if kcs[-1] == KC - 1:
                del psv_tiles[j]
                nc.vector.tensor_copy(
                    out=v4[:, j, :, 0:DH],
                    in_=psv.rearrange("p (h e) -> p h e", h=HL))
        return (600, "v", emit)

    def v_unit(j, pool=None, tag=None):
        parts = [v_part(j, [0, 1, 2], pool, tag), v_part(j, [3, 4, 5], pool, tag)]

        def emit():
            for _, _, fn in parts:
                fn()
        return (1100, "v", emit)

    yst_tiles = {}
    pc_tiles = {}

    def ctx_part(hp, c, a, exp_pair, pieces, first, last, est):
        """A slice of a ctx accumulation; finer filler granularity keeps the
        PE pacing against the exp chain tight (big units overshoot the
        per-bin budget and stall the sg ping-pong)."""
        def emit():
            key = (hp, c, a)
            if key not in pc_tiles:
                pc_tiles[key] = cx.tile([DH + 1, 512], F32, tag=f"cx{a}",
                                        name=f"pc{hp}{c}{a}")
            pc = pc_tiles[key]
            for idx, (j, s0, w, off) in enumerate(pieces):
                nc.tensor.matmul(
                    out=pc[:, s0 - 512 * c: s0 - 512 * c + w],
                    lhsT=v4[:, j, 2 * hp + a, :],
                    rhs=exp_pair[:, a, off:off + w],
                    start=(first and idx == 0),
                    stop=(last and idx == len(pieces) - 1))
            if not last:
                return
            del pc_tiles[key]
            ykey = (hp, c)
            if ykey not in yst_tiles:
                yst_tiles[ykey] = yst.tile([DH + 1, 2, 512], BF16, tag="yst",
                                           name=f"yt{hp}{c}")
            yt = yst_tiles[ykey]
            nc.vector.tensor_copy(out=yt[:, a, :], in_=pc)
            if a == 1:
                nc.sync.dma_start(
                    out=y[:, 2 * hp:2 * hp + 2, 512 * c:512 * (c + 1)], in_=yt)
        return (est, "ctx", emit)

    def ctx_units(hp, c, a, exp_pair):
        pieces = CTX[c]
        if c == 0:
            return [ctx_part(hp, c, a, exp_pair, pieces, True, True, 600)]
        return [ctx_part(hp, c, a, exp_pair, pieces[:4], True, False, 700),
                ctx_part(hp, c, a, exp_pair, pieces[4:], False, True, 800)]

    def scores_bin(hp, b, exp_pair):
        ps = sg.tile([128, 2, 512], F32, tag="sg", name=f"sg{hp}{b}")
        pieces = BINS[b]
        # piece-major, heads adjacent: the two heads' K=64 matmuls occupy PE
        # row groups 0/64 and different PSUM banks, so adjacent pairs execute
        # concurrently on the array (observed ~2x on the scores phase)
        for idx, (j, s0, w, o) in enumerate(pieces):
            for a in (0, 1):
                nc.tensor.matmul(
                    out=ps[:, a, o:o + w],
                    lhsT=kT[64 * a:64 * a + 64, hp, 128 * j:128 * (j + 1)],
                    rhs=qT[64 * a:64 * a + 64, hp, s0:s0 + w],
                    start=(idx == 0), stop=(idx == len(pieces) - 1))
        nc.scalar.activation(
            out=exp_pair[:, :, 512 * b:512 * (b + 1)],
            in_=ps,
            func=mybir.ActivationFunctionType.Exp,
            scale=1.0 / np.sqrt(DH))

    def emit_masks(hp, group, exp_pair):
        # DVE, not GpSimd: gpsimd pays a LIBRARY_RELOAD ucode swap per op type
        for a in (0, 1):
            for j in MASK_GROUPS[group]:
                sl = exp_pair[:, a, DIAG_OFF[j]:DIAG_OFF[j] + 128]
                nc.vector.tensor_mul(sl, sl, im_sb)

    def warm_fill(n, name):
        # sacrificial matmuls into an idle sg bank: keep PE busy (and its
        # clock ramp alive) across input-DMA arrival gaps
        wt = sg.tile([128, 2, 512], F32, tag="sg", name=name)
        for i in range(n):
            nc.tensor.matmul(out=wt[:, 0, 0:128], lhsT=wsrc, rhs=wsrc,
                             start=(i == 0), stop=(i == n - 1))

    # ---- schedule ----
    # pair-0 projections emitted directly, q/k interleaved per chunk pair so
    # unit consumption matches DMA chunk arrival; k units run on the cx banks
    # so they don't serialize against the q units' PSUM->SBUF casts.  One v
    # unit covers the kT cast latency before bin 0.
    for kcs in ([0, 1], [2, 3], [4, 5]):
        proj_qk_unit(0, 0, kcs)[2]()
        proj_qk_unit(0, 1, kcs, pool=cx, tags=("cx0", "cx1"))[2]()
        if kcs[0] < 4:
            warm_fill(10, f"wf{kcs[0]}")
    v_unit(0, pool=pj, tag="pj0")[2]()
    for j in range(1, 8):
        fillers.append(v_part(j, [0, 1, 2]))
        fillers.append(v_part(j, [3, 4, 5]))

    for hp in range(NPAIR):
        if hp + 1 < NPAIR:
            fillers.extend(proj_pair_units(hp + 1))
        exp_pair = ex.tile([128, 2, EXP_COLS], BF16, tag="exp", name=f"exp{hp}")
        for b in range(len(BINS)):
            scores_bin(hp, b, exp_pair)
            if b == 2:
                emit_masks(hp, 2, exp_pair)
                for a in (0, 1):
                    fillers.extend(ctx_units(hp, 0, a, exp_pair))
            elif b == 4:
                emit_masks(hp, 4, exp_pair)
            elif b == 8:
                # ctx c1 reads bin 8's exp, so it may only be EMITTED after
                # scores_bin(hp, 8): deps come from program order, and a unit
                # emitted before its producer silently reads stale data.
                for a in (0, 1):
                    fillers.extend(ctx_units(hp, 1, a, exp_pair))
            emit_fillers(1200)
        # next pair's projections must be fully emitted before its scores
        # (their PSUM->SBUF copies feed the scores matmuls)
        keep = []
        for u in fillers:
            if u[1] in ("proj", "v"):
                u[2]()
            else:
                keep.append(u)
        fillers[:] = keep

    while fillers:
        _, _, fn = fillers.pop(0)
        fn()


_PROGRAM = None
_PROGRAM_LOCK = threading.Lock()


def _get_program() -> bass.Bass:
    global _PROGRAM
    with _PROGRAM_LOCK:
        if _PROGRAM is None:
            nc = bacc.Bacc(None, target_bir_lowering=False)
            xT = nc.declare_dram_parameter("xT", [D, S], BF16, isOutput=False)
            wqkv = nc.declare_dram_parameter("wqkv", [KC, 128, 3, HL * DH], BF16,
                                             isOutput=False)
            im = nc.declare_dram_parameter("im", [128, 128], BF16, isOutput=False)
            y = nc.declare_dram_parameter("y_aug", [DH + 1, HL, S], BF16,
                                          isOutput=True)
            with tile.TileContext(nc) as tc, ExitStack() as ctx:
                _emit_kernel(ctx, tc, xT, wqkv, im, y)
            nc.finalize()
            _PROGRAM = nc
    return _PROGRAM


def make_in_maps(x, Wq, Wk, Wv):
    """Per-core input dicts: batch b=core//2, heads (core%2)*6..+6."""
    bf = ml_dtypes.bfloat16
    t = np.arange(128)
    im = (t[None, :] >= t[:, None]).astype(bf)  # 1 where s_rel >= t_rel
    in_maps = []
    for core in range(NCORES):
        b, hs = core // 2, (core % 2) * HL
        xTc = np.ascontiguousarray(np.asarray(x[b]).T.astype(bf))
        # wqkv[kc, p, t, h*64+e] = W_t[hs+h, kc*128+p, e]
        w = np.stack([np.asarray(W[hs:hs + HL]) for W in (Wq, Wk, Wv)], axis=0)
        # w: [3, HL, D, DH] -> [KC, 128, 3, HL*DH]
        w = w.transpose(2, 0, 1, 3).reshape(KC, 128, 3, HL, DH)
        w = np.ascontiguousarray(w.reshape(KC, 128, 3, HL * DH).astype(bf))
        in_maps.append({"xT": xTc, "wqkv": w, "im": im})
    return in_maps


def assemble_output(per_core_results):
    y_full = np.zeros((B, S, H * DH), np.float32)
    for core in range(NCORES):
        ya = per_core_results[core]["y_aug"].astype(np.float32)  # [65, 6, 1024]
        b, hs = core // 2, (core % 2) * HL
        ctxs = ya[0:DH] / ya[DH:DH + 1]                  # [64, 6, 1024]
        y_full[b, :, hs * DH:(hs + HL) * DH] = (
            ctxs.transpose(2, 1, 0).reshape(S, HL * DH))
    return y_full


def kernel(x, Wq, Wk, Wv):
    nc = _get_program()
    in_maps = make_in_maps(x, Wq, Wk, Wv)
    res = run_bass_kernel_spmd(nc, in_maps, core_ids=list(range(NCORES)))
    return assemble_output(res.results)
